# revision 24
# baseline (speedup 1.0000x reference)
"""HEX loss kernel v4: full-stream + host one-hot mask.

loss = mean_b softplus(-fs[b, labels[b]]).

Per core: stream fs (4 MB f32, cast to bf16 in SWDGE DMA) and a one-hot
mask (1 MB u8, cast to bf16 in DMA) in 4 pipelined groups. DVE does one
dense flat bf16 multiply per group (2x perf mode) and one grouped
reduce-add (sel = fs[b, lab]); ACT computes softplus(-sel) =
Ln(1+Exp(-sel)) with accum_out, then issues the output DMA itself.
No completion wait on the final DMA (block-end drains fence it).
"""

import numpy as np

B = 32768
V = 256
N_CORES = 8
BL = B // N_CORES  # 4096
P = 128
NT = BL // P       # 32
NG = 4
RPP = 8

_CACHE = {}
FINAL_WAIT = False  # keep the explicit out-DMA completion wait?


def _build():
    from contextlib import ExitStack

    import concourse.bass as bass  # noqa
    from concourse import bacc, mybir

    f32 = mybir.dt.float32
    bf16 = mybir.dt.bfloat16
    u8 = mybir.dt.uint8
    Alu = mybir.AluOpType
    Act = mybir.ActivationFunctionType

    nc = bacc.Bacc(
        "TRN2",
        target_bir_lowering=False,
        debug=False,
        enable_asserts=True,
        num_devices=N_CORES,
    )

    fs_d = nc.dram_tensor("fs", [BL, V], f32, kind="ExternalInput").ap()
    mk_d = nc.dram_tensor("msk", [BL, V], u8, kind="ExternalInput").ap()
    out_d = nc.dram_tensor("out", [P, 1], f32, kind="ExternalOutput").ap()

    fs_view = fs_d.rearrange("(g p j) v -> g p (j v)", g=NG, p=P, j=RPP)
    mk_view = mk_d.rearrange("(g p j) v -> g p (j v)", g=NG, p=P, j=RPP)

    with ExitStack() as ctx:
        fs16 = [
            ctx.enter_context(nc.sbuf_tensor(f"fs16_{g}", [P, RPP * V], bf16))
            for g in range(NG)
        ]
        mk16 = [
            ctx.enter_context(nc.sbuf_tensor(f"mk16_{g}", [P, RPP * V], bf16))
            for g in range(NG)
        ]
        uu = [
            ctx.enter_context(nc.sbuf_tensor(f"uu{g}", [P, RPP * V], bf16))
            for g in range(NG)
        ]
        sel = ctx.enter_context(nc.sbuf_tensor([P, NT], f32))
        u2 = ctx.enter_context(nc.sbuf_tensor([P, NT], f32))
        sp = ctx.enter_context(nc.sbuf_tensor([P, NT], f32))
        acc = ctx.enter_context(nc.sbuf_tensor([P, 1], f32))

        sem_g = [ctx.enter_context(nc.semaphore(f"s_g{g}")) for g in range(NG)]
        sem_sel = ctx.enter_context(nc.semaphore("s_sel"))
        sem_out = ctx.enter_context(nc.semaphore("s_out"))

        blk = ctx.enter_context(nc.Block())

        uu3 = [u.ap().rearrange("p (j v) -> p j v", j=RPP) for u in uu]

        @blk.gpsimd
        def _(g_eng):
            for g in range(NG):
                g_eng.dma_start(out=fs16[g].ap(), in_=fs_view[g]).then_inc(
                    sem_g[g], 16
                )
                g_eng.dma_start(out=mk16[g].ap(), in_=mk_view[g]).then_inc(
                    sem_g[g], 16
                )

        @blk.vector
        def _(v_eng):
            for g in range(NG):
                v_eng.wait_ge(sem_g[g], 32)
                # dense flat bf16 multiply -> 2x DVE mode
                v_eng.tensor_tensor(
                    uu[g].ap(), mk16[g].ap(), fs16[g].ap(), Alu.mult
                )
                v_eng.drain()
                v_eng.tensor_reduce(
                    sel.ap()[:, g * RPP : (g + 1) * RPP],
                    uu3[g],
                    axis=mybir.AxisListType.X,
                    op=Alu.add,
                )
                v_eng.drain()
            v_eng.engine_nop().then_inc(sem_sel, 1)

        @blk.scalar
        def _(a_eng):
            a_eng.add_instruction(
                mybir.InstLoadActFuncSet(
                    name=nc.get_next_instruction_name(),
                    act_func_set_id=6,
                    ins=[],
                    outs=[],
                )
            )
            a_eng.wait_ge(sem_sel, 1)
            a_eng.activation(u2.ap(), sel.ap(), Act.Exp, scale=-1.0)
            a_eng.drain()
            a_eng.activation(
                sp.ap(), u2.ap(), Act.Ln, bias=1.0, accum_out=acc.ap()
            )
            a_eng.drain()
            a_eng.dma_start(out=out_d, in_=acc.ap()).then_inc(sem_out, 16)
            if FINAL_WAIT:
                a_eng.wait_ge(sem_out, 16)

    nc.compile()
    return nc


def _get_nc():
    if "nc" not in _CACHE:
        _CACHE["nc"] = _build()
    return _CACHE["nc"]


def _shard_inputs(fs, labels):
    fs = np.ascontiguousarray(np.asarray(fs, dtype=np.float32))
    labels = np.asarray(labels).astype(np.int64)
    cols = np.arange(V, dtype=np.int64)
    in_maps = []
    for c in range(N_CORES):
        fs_loc = fs[c * BL : (c + 1) * BL]
        lab_loc = labels[c * BL : (c + 1) * BL]
        msk = (cols[None, :] == lab_loc[:, None]).astype(np.uint8)
        in_maps.append({"fs": fs_loc, "msk": np.ascontiguousarray(msk)})
    return in_maps


def kernel(fs, labels, _trace=False, _trace_kwargs=None):
    from concourse.bass_utils import run_bass_kernel_spmd

    nc = _get_nc()
    in_maps = _shard_inputs(fs, labels)
    res = run_bass_kernel_spmd(
        nc,
        in_maps,
        core_ids=list(range(N_CORES)),
        trace=_trace,
        **(_trace_kwargs or {}),
    )
    total = np.float64(0.0)
    for c in range(N_CORES):
        total += res.results[c]["out"].astype(np.float64).sum()
    loss = total / np.float64(B)
    if _trace:
        return np.float64(loss), res
    return np.asarray(loss, dtype=np.float64)
